# revision 8
# baseline (speedup 1.0000x reference)
"""Trainium2 Bass kernel for EnhancedLinkPredictor (GNN common-neighbor link prediction).

Math (per prediction edge e=(s,d)):
  shared_ddi = adj_ddi[s] & adj_ddi[d]          (drug-drug, N_D=8192)
  cn_ddi     = (shared_ddi @ z_drug)  / max(|shared_ddi|, 1)
  shared_dp  = adj_dp[s]  & adj_dp[d]           (drug-protein, N_P=4096)
  cn_prot    = (shared_dp @ z_protein) / max(|shared_dp|, 1)
  pair  = [z_drug[s], z_drug[d], cn_ddi, cn_prot]   (256)
  out   = sigmoid(relu(pair @ W1 + b1) @ W2 + b2)

Device strategy (8 cores, data-parallel over the 16384 pred edges, 2048/core):
  - One merged table row per drug: [ddi nibbles 4096B | z bf16 256B | dp
    nibbles 2048B] = 25 chunks x 256B, compacted per core to the <=4096 rows
    it touches. ONE dma_gather(transpose=True) per 256-edge block fetches
    s+d rows (512 idxs): partition p of chunk c holds bytes 2p..2p+1, i.e.
    packed entries k = 512c + 4p + {0..3}.
  - Adjacency nibbles are BIT-coded: entry 2j -> 0x08 (fp8 2^-6), entry
    2j+1 -> 0x10 (fp8 2^-5). One scalar_tensor_tensor per parity computes
    (s & code) & d, yielding the fp8 intersection plane directly (no
    separate AND pass). The 2x scale gap between parities is folded into
    the Z packing (m=0 rows x4, m=1 rows x2 => uniform product scale 1/16).
  - cn matmuls run fp8 DoubleRow with a 128-wide stationary holding BOTH
    limbs: cols [0:64]=e4m3 hi of z*s_m, col 64 = count (s_m), cols
    [65:128] = e4m3 lo limb of dims 0..62 (dim 63 is hi-only; adds ~4e-3
    rel err, still 2x under the gate). Matmul cost is N-cols only, so the
    second limb is FREE. The hi+lo summation happens inside the MLP W1
    matmul via duplicated W1 rows (k=128 per block costs the same as 64).
  - Normalize: counts sit in PSUM row 64; gpsimd.partition_broadcast
    spreads them to 128 partitions, then DVE max(.,1/16) +
    reciprocal_approx_fast + one multiply produce the MLP rhs.
"""

import numpy as np
import ml_dtypes
from contextlib import ExitStack

import concourse.bass as bass
import concourse.bacc as bacc
import concourse.mybir as mybir
import concourse.tile as tile

N_D, N_P = 8192, 4096
D_DIM, HID = 64, 128
E_PRED = 16384
N_CORES = 8
E_LOC = E_PRED // N_CORES          # 2048 edges per core
U_PAD = 4096                       # compacted adjacency row count

C_A = N_D // 512                   # 16 ddi chunks (512 entries each)
C_P = N_P // 512                   # 8 dp chunks
C_TOT = C_A + 1 + C_P              # 25 chunks per merged row
ROW_U16 = C_TOT * 128              # 3200 u16 = 6400 B per row
G = 256                            # edges per gather call (512 idxs)
N_CALLS = E_LOC // G               # 8 calls
N_ET = E_LOC // 512                # 4 supertiles of 512 edges
IDX_COLS = N_CALLS * (2 * G // 16)  # 256

CODE_LO, CODE_HI = 0x08, 0x10      # fp8 e4m3: 2^-6 and 2^-5
SCALE_M = (4.0, 2.0)               # z premultiplier per parity m
S_OUT = 2.0 ** -4                  # uniform (code * scale) product = 1/16

FP8 = ml_dtypes.float8_e4m3
BF16 = ml_dtypes.bfloat16


def _pack_z(z: np.ndarray):
    """z [K, 64] f32 -> [128, (K/256)*2*128] uint8 fp8 DoubleRow lhsT blocks.
    Group g = c*2 + m holds rows k = 512c + 4p + m + 2i at (partition p,
    sub-row i), scaled by SCALE_M[m]. Cols: [0:64] hi limb, 64 = count
    (SCALE_M[m]), [65:128] lo limb of dims 0..62."""
    K = z.shape[0]
    n512 = K // 512
    p = np.arange(128)[:, None]
    i = np.arange(2)[None, :]
    out = np.empty((2 * n512, 128, 2, 128), dtype=np.uint8)
    for c in range(n512):
        for m in range(2):
            ks = 512 * c + 4 * p + m + 2 * i          # [128, 2]
            zsc = z[ks].astype(np.float32) * SCALE_M[m]  # [128, 2, 64]
            hi8 = zsc.astype(FP8)
            lo8 = (zsc - hi8.astype(np.float32)).astype(FP8)
            blk = np.zeros((128, 2, 128), dtype=np.uint8)
            blk[..., 0] = np.float32(SCALE_M[m]).astype(FP8).view(np.uint8)
            blk[..., 1:65] = hi8.view(np.uint8)
            blk[..., 65:128] = lo8.view(np.uint8)[..., :63]
            out[c * 2 + m] = blk
    return np.ascontiguousarray(out.transpose(1, 0, 2, 3).reshape(128, -1))


def _wrap_idxs(idx: np.ndarray):
    """[n] int -> [128, n/16] int16 wrapped (j -> [j%16, j//16]) + 8x replicated."""
    n = idx.shape[0]
    w = np.zeros((16, n // 16), dtype=np.int16)
    w[np.arange(n) % 16, np.arange(n) // 16] = idx.astype(np.int16)
    return np.tile(w, (8, 1))


def build_body(tc, t):
    """Emit the per-core program. t: dict name -> AP of DRAM tensors."""
    nc = tc.nc
    dt = mybir.dt
    with ExitStack() as ctx:
        const = ctx.enter_context(tc.tile_pool(name="const", bufs=1))
        gpool = ctx.enter_context(tc.tile_pool(name="gath", bufs=2))
        mka = ctx.enter_context(tc.tile_pool(name="mska", bufs=2))
        mkb = ctx.enter_context(tc.tile_pool(name="mskb", bufs=2))
        tails = ctx.enter_context(tc.tile_pool(name="tails", bufs=2))
        pairp = ctx.enter_context(tc.tile_pool(name="pair", bufs=1))
        psum = ctx.enter_context(tc.tile_pool(name="ps", bufs=8, space="PSUM"))

        # idx on the sync queue (gathers depend on it); bulk constants on the
        # scalar HWDGE queue so they don't delay the first gather.
        idxt = const.tile([128, IDX_COLS], dt.int16)
        nc.sync.dma_start(idxt[:], t["IDX"][:, :])

        zd = const.tile([128, 2 * C_A * 2 * 128], dt.uint8)
        nc.scalar.dma_start(zd[:], t["ZD"][:, :])
        zp = const.tile([128, 2 * C_P * 2 * 128], dt.uint8)
        nc.scalar.dma_start(zp[:], t["ZP"][:, :])
        w1t = const.tile([128, 4 * HID], dt.uint16)
        nc.scalar.dma_start(w1t[:], t["W1"][:, :])
        w2t = const.tile([128, 1], dt.uint16)
        nc.scalar.dma_start(w2t[:], t["W2"][:, :])
        b1t = const.tile([128, 1], dt.float32)
        nc.scalar.dma_start(b1t[:], t["B1"][:, :])
        b2t = const.tile([1, 1], dt.float32)
        nc.scalar.dma_start(b2t[:], t["B2"][:, :])

        zsrc = pairp.tile([128, E_LOC], dt.uint16)
        zdst = pairp.tile([128, E_LOC], dt.uint16)

        codes = const.tile([128, 2], dt.uint32)
        nc.vector.memset(codes[:, 0:1], 0x08080808)
        nc.vector.memset(codes[:, 1:2], 0x10101010)

        zd8 = zd[:].bitcast(dt.float8e4).rearrange(
            "p (g two m) -> p g two m", g=2 * C_A, two=2
        )
        zp8 = zp[:].bitcast(dt.float8e4).rearrange(
            "p (g two m) -> p g two m", g=2 * C_P, two=2
        )

        st_state = {}

        def gather_and_mask(et):
            """Gathers + z copies + fp8 mask planes for supertile et."""
            mska_t = mka.tile([128, 2 * C_A * 2 * G * 2 // 2], dt.uint16,
                              tag="a", name=f"mka{et}")
            mskb_t = mkb.tile([128, 2 * C_P * 2 * G * 2 // 2], dt.uint16,
                              tag="b", name=f"mkb{et}")
            oa = mska_t[:].bitcast(dt.uint32).rearrange(
                "p (m c s w) -> p m c s w", m=2, c=C_A, s=2
            )
            ob = mskb_t[:].bitcast(dt.uint32).rearrange(
                "p (m c s w) -> p m c s w", m=2, c=C_P, s=2
            )
            W = G // 2  # u32 words per chunk per endpoint half
            for sub in range(2):
                g = 2 * et + sub
                gt = gpool.tile([128, C_TOT * 2 * G], dt.uint16, tag="gt")
                gv = gt[:].rearrange("p (c i) -> p c i", c=C_TOT)
                nc.gpsimd.dma_gather(
                    out_ap=gv,
                    in_ap=t["A"][:, :],
                    idxs_ap=idxt[:, g * 32:(g + 1) * 32],
                    num_idxs=2 * G,
                    num_idxs_reg=2 * G,
                    elem_size=ROW_U16,
                    elem_step=ROW_U16,
                    transpose=True,
                    single_packet=False,
                )
                # z chunk -> pair^T rows (s first half, d second half)
                nc.scalar.copy(
                    zsrc[:].bitcast(dt.bfloat16)[:, G * g:G * (g + 1)],
                    gv[:, C_A, 0:G].bitcast(dt.bfloat16),
                )
                nc.scalar.copy(
                    zdst[:].bitcast(dt.bfloat16)[:, G * g:G * (g + 1)],
                    gv[:, C_A, G:2 * G].bitcast(dt.bfloat16),
                )
                g32 = gt[:].bitcast(dt.uint32).rearrange(
                    "p (c w) -> p c w", c=C_TOT
                )
                for m in range(2):
                    nc.vector.scalar_tensor_tensor(
                        oa[:, m, :, sub, :],
                        g32[:, 0:C_A, 0:W],
                        codes[:, m:m + 1],
                        g32[:, 0:C_A, W:2 * W],
                        mybir.AluOpType.bitwise_and,
                        mybir.AluOpType.bitwise_and,
                    )
                    nc.vector.scalar_tensor_tensor(
                        ob[:, m, :, sub, :],
                        g32[:, C_A + 1:C_TOT, 0:W],
                        codes[:, m:m + 1],
                        g32[:, C_A + 1:C_TOT, W:2 * W],
                        mybir.AluOpType.bitwise_and,
                        mybir.AluOpType.bitwise_and,
                    )
            st_state[et] = (mska_t, mskb_t)

        def cn_matmuls(et):
            mska_t, mskb_t = st_state[et]
            psa = psum.tile([128, 512], dt.float32, tag="ps", name=f"psa{et}")
            psb = psum.tile([128, 512], dt.float32, tag="ps", name=f"psb{et}")
            ma = mska_t[:].bitcast(dt.float8e4).rearrange(
                "p (m c i two) -> p c m two i", m=2, c=C_A, two=2
            )
            mb = mskb_t[:].bitcast(dt.float8e4).rearrange(
                "p (m c i two) -> p c m two i", m=2, c=C_P, two=2
            )
            for c in range(C_A):
                for m in range(2):
                    nc.tensor.matmul(
                        psa[:],
                        zd8[:, c * 2 + m],
                        ma[:, c, m],
                        start=(c == 0 and m == 0),
                        stop=(c == C_A - 1 and m == 1),
                        perf_mode=mybir.MatmulPerfMode.DoubleRow,
                    )
            for c in range(C_P):
                for m in range(2):
                    nc.tensor.matmul(
                        psb[:],
                        zp8[:, c * 2 + m],
                        mb[:, c, m],
                        start=(c == 0 and m == 0),
                        stop=(c == C_P - 1 and m == 1),
                        perf_mode=mybir.MatmulPerfMode.DoubleRow,
                    )
            st_state[et] = (psa, psb)

        def tail(et):
            """Normalize + MLP + output for supertile et."""
            psa, psb = st_state.pop(et)
            # counts live in PSUM row 0; clamp+invert them on lane 0 in
            # SBUF (gpsimd cannot read PSUM), then broadcast to all lanes.
            cnt = tails.tile([1, 1024], dt.float32, tag="cnt")
            nc.vector.tensor_scalar_max(cnt[0:1, 0:512], psa[0:1, :], S_OUT)
            nc.vector.tensor_scalar_max(cnt[0:1, 512:1024], psb[0:1, :], S_OUT)
            rec = tails.tile([1, 1024], dt.float32, tag="rec")
            nc.vector.reciprocal_approx_fast(rec[:], cnt[:])
            rhs = {}
            for rel, ps, c0 in (("a", psa, 0), ("b", psb, 512)):
                bc = tails.tile([128, 512], dt.float32, tag=f"bc{rel}")
                nc.gpsimd.partition_broadcast(bc[:], rec[0:1, c0:c0 + 512])
                rh = tails.tile([128, 512], dt.bfloat16, tag=f"rh{rel}")
                nc.vector.tensor_tensor(
                    rh[:], ps[:], bc[:], mybir.AluOpType.mult
                )
                rhs[rel] = rh
            hps = psum.tile([HID, 512], dt.float32, tag="ps")
            rhs_chunks = (
                zsrc[:].bitcast(dt.bfloat16)[:, 512 * et:512 * (et + 1)],
                zdst[:].bitcast(dt.bfloat16)[:, 512 * et:512 * (et + 1)],
                rhs["a"][:],
                rhs["b"][:],
            )
            for j, r in enumerate(rhs_chunks):
                nc.tensor.matmul(
                    hps[:],
                    w1t[:].bitcast(dt.bfloat16)[:, HID * j:HID * (j + 1)],
                    r,
                    start=(j == 0),
                    stop=(j == 3),
                )
            hsb = tails.tile([HID, 512], dt.bfloat16, tag="h")
            nc.scalar.activation(
                hsb[:], hps[:], mybir.ActivationFunctionType.Relu, bias=b1t[:, 0:1]
            )
            lps = psum.tile([1, 512], dt.float32, tag="ps")
            nc.tensor.matmul(
                lps[:], w2t[:].bitcast(dt.bfloat16), hsb[:], start=True, stop=True
            )
            osb = tails.tile([1, 512], dt.float32, tag="osb")
            nc.scalar.activation(
                osb[:],
                lps[:],
                mybir.ActivationFunctionType.Sigmoid,
                bias=b2t[:, 0:1],
            )
            nc.sync.dma_start(t["OUT"][:, 512 * et:512 * (et + 1)], osb[:])

        for et in range(N_ET):
            gather_and_mask(et)
            cn_matmuls(et)
            if et > 0:
                tail(et - 1)
        tail(N_ET - 1)


def build_program():
    nc = bacc.Bacc("TRN2", target_bir_lowering=False)
    dt = mybir.dt
    t = {
        "A": nc.dram_tensor("A", [U_PAD, ROW_U16], dt.uint16, kind="ExternalInput").ap(),
        "IDX": nc.dram_tensor("IDX", [128, IDX_COLS], dt.int16, kind="ExternalInput").ap(),
        "ZD": nc.dram_tensor(
            "ZD", [128, 2 * C_A * 2 * 128], dt.uint8, kind="ExternalInput"
        ).ap(),
        "ZP": nc.dram_tensor(
            "ZP", [128, 2 * C_P * 2 * 128], dt.uint8, kind="ExternalInput"
        ).ap(),
        "W1": nc.dram_tensor("W1", [128, 4 * HID], dt.uint16, kind="ExternalInput").ap(),
        "B1": nc.dram_tensor("B1", [HID, 1], dt.float32, kind="ExternalInput").ap(),
        "W2": nc.dram_tensor("W2", [HID, 1], dt.uint16, kind="ExternalInput").ap(),
        "B2": nc.dram_tensor("B2", [1, 1], dt.float32, kind="ExternalInput").ap(),
        "OUT": nc.dram_tensor("OUT", [1, E_LOC], dt.float32, kind="ExternalOutput").ap(),
    }
    with tile.TileContext(nc) as tc:
        build_body(tc, t)
    nc.compile()
    return nc


def host_prep(z_drug, z_protein, ddi_ei, dp_ei, pred_ei, W1, b1, W2, b2):
    """Build the 8 per-core input maps (all numpy, no device work)."""
    z_drug = np.asarray(z_drug, np.float32)
    z_protein = np.asarray(z_protein, np.float32)
    ddi_ei = np.asarray(ddi_ei, np.int64)
    dp_ei = np.asarray(dp_ei, np.int64)
    pred_ei = np.asarray(pred_ei, np.int64)

    A_ddi = np.zeros((N_D, N_D), dtype=np.uint8)
    A_ddi[ddi_ei[0], ddi_ei[1]] = 1
    A_ddi[ddi_ei[1], ddi_ei[0]] = 1
    A_dp = np.zeros((N_D, N_P), dtype=np.uint8)
    A_dp[dp_ei[0], dp_ei[1]] = 1
    # bit-coded nibble pack: entry 2j -> 0x08 (low), entry 2j+1 -> 0x10 (high)
    A_ddi_nib = (A_ddi[:, 0::2] * CODE_LO) | (A_ddi[:, 1::2] * CODE_HI)
    A_dp_nib = (A_dp[:, 0::2] * CODE_LO) | (A_dp[:, 1::2] * CODE_HI)

    zb_full = np.zeros((N_D, 128), dtype=np.float32)
    zb_full[:, :D_DIM] = z_drug
    zb_bytes = zb_full.astype(BF16).view(np.uint8)  # [N_D, 256]

    ZD = _pack_z(z_drug)
    ZP = _pack_z(z_protein)

    # W1 blocks [128, HID] bf16: rows 0:64 = hi-limb dims, row 64 = 0 (count
    # row), rows 65:128 = lo-limb dims 0..62 (duplicated weights sum limbs).
    W1f = np.asarray(W1, np.float32)  # [256, HID]
    blocks = np.zeros((4, 128, HID), dtype=np.float32)
    blocks[0, 0:64] = W1f[0:64]
    blocks[1, 0:64] = W1f[64:128]
    blocks[2, 1:65] = W1f[128:192]
    blocks[2, 65:128] = W1f[128:191]
    blocks[3, 1:65] = W1f[192:256]
    blocks[3, 65:128] = W1f[192:255]
    W1p = np.ascontiguousarray(
        blocks.astype(BF16).view(np.uint16).transpose(1, 0, 2).reshape(128, 4 * HID)
    )
    B1 = np.asarray(b1, np.float32).reshape(HID, 1)
    W2p = np.asarray(W2, np.float32).reshape(HID, 1).astype(BF16).view(np.uint16)
    B2 = np.asarray(b2, np.float32).reshape(1, 1)

    in_maps = []
    for core in range(N_CORES):
        s = pred_ei[0, core * E_LOC:(core + 1) * E_LOC]
        d = pred_ei[1, core * E_LOC:(core + 1) * E_LOC]
        rows = np.unique(np.concatenate([s, d]))
        nu = rows.shape[0]
        assert nu <= U_PAD
        remap_s = np.searchsorted(rows, s).astype(np.int16)
        remap_d = np.searchsorted(rows, d).astype(np.int16)
        A = np.zeros((U_PAD, 2 * ROW_U16), dtype=np.uint8)
        A[:nu, 0:N_D // 2] = A_ddi_nib[rows]
        A[:nu, N_D // 2:N_D // 2 + 256] = zb_bytes[rows]
        A[:nu, N_D // 2 + 256:] = A_dp_nib[rows]

        cols = []
        for g in range(N_CALLS):
            cols.append(
                _wrap_idxs(
                    np.concatenate(
                        [remap_s[G * g:G * (g + 1)], remap_d[G * g:G * (g + 1)]]
                    )
                )
            )
        idx = np.concatenate(cols, axis=1)
        assert idx.shape == (128, IDX_COLS)

        in_maps.append(
            {
                "A": A.view(np.uint16),
                "IDX": idx,
                "ZD": ZD,
                "ZP": ZP,
                "W1": W1p,
                "B1": B1,
                "W2": W2p,
                "B2": B2,
            }
        )
    return in_maps


def kernel(z_drug, z_protein, ddi_ei, dp_ei, pred_ei, W1, b1, W2, b2, _profile=None):
    from concourse.bass_utils import run_bass_kernel_spmd

    in_maps = host_prep(z_drug, z_protein, ddi_ei, dp_ei, pred_ei, W1, b1, W2, b2)
    nc = build_program()
    res = run_bass_kernel_spmd(
        nc,
        in_maps,
        core_ids=list(range(N_CORES)),
        **({} if _profile is None else _profile),
    )
    if _profile is not None:
        kernel.last_results = res
    out = np.concatenate([r["OUT"].reshape(-1) for r in res.results])
    return out.astype(np.float32)


# revision 15
# speedup vs baseline: 1.1681x; 1.1681x over previous
"""Trainium2 Bass kernel for EnhancedLinkPredictor (GNN common-neighbor link prediction).

Math (per prediction edge e=(s,d)):
  shared_ddi = adj_ddi[s] & adj_ddi[d]          (drug-drug, N_D=8192)
  cn_ddi     = (shared_ddi @ z_drug)  / max(|shared_ddi|, 1)
  shared_dp  = adj_dp[s]  & adj_dp[d]           (drug-protein, N_P=4096)
  cn_prot    = (shared_dp @ z_protein) / max(|shared_dp|, 1)
  pair  = [z_drug[s], z_drug[d], cn_ddi, cn_prot]   (256)
  out   = sigmoid(relu(pair @ W1 + b1) @ W2 + b2)

Device strategy (8 cores, data-parallel over the 16384 pred edges, 2048/core):
  - One merged table row per drug: [ddi nibbles 4096B | z bf16 256B | dp
    nibbles 2048B] = 25 chunks x 256B, compacted per core to the <=4096 rows
    it touches. ONE dma_gather(transpose=True) per 256-edge block fetches
    s+d rows (512 idxs): partition p of chunk c holds bytes 2p..2p+1, i.e.
    packed entries k = 512c + 4p + {0..3}.
  - Adjacency nibbles are BIT-coded: entry 2j -> 0x08 (fp8 2^-6), entry
    2j+1 -> 0x10 (fp8 2^-5). One scalar_tensor_tensor per parity computes
    (s & code) & d, yielding the fp8 intersection plane directly (no
    separate AND pass). The 2x scale gap between parities is folded into
    the Z packing (m=0 rows x4, m=1 rows x2 => uniform product scale 1/16).
  - cn matmuls run fp8 DoubleRow with a 128-wide stationary holding BOTH
    limbs: cols [0:64]=e4m3 hi of z*s_m, col 64 = count (s_m), cols
    [65:128] = e4m3 lo limb of dims 0..62 (dim 63 is hi-only; adds ~4e-3
    rel err, still 2x under the gate). Matmul cost is N-cols only, so the
    second limb is FREE. The hi+lo summation happens inside the MLP W1
    matmul via duplicated W1 rows (k=128 per block costs the same as 64).
  - Normalize: counts sit in PSUM row 64; gpsimd.partition_broadcast
    spreads them to 128 partitions, then DVE max(.,1/16) +
    reciprocal_approx_fast + one multiply produce the MLP rhs.
"""

import numpy as np
import ml_dtypes
from contextlib import ExitStack

import concourse.bass as bass
import concourse.bacc as bacc
import concourse.mybir as mybir
import concourse.tile as tile

N_D, N_P = 8192, 4096
D_DIM, HID = 64, 128
E_PRED = 16384
N_CORES = 8
E_LOC = E_PRED // N_CORES          # 2048 edges per core
U_PAD = 4096                       # compacted adjacency row count

C_A = N_D // 512                   # 16 ddi chunks (512 entries each)
C_P = N_P // 512                   # 8 dp chunks
C_TOT = C_A + 1 + C_P              # 25 chunks per merged row
ROW_U16 = C_TOT * 128              # 3200 u16 = 6400 B per row
G = 256                            # edges per gather call (512 idxs)
N_CALLS = E_LOC // G               # 8 calls
N_ET = E_LOC // 512                # 4 supertiles of 512 edges
IDX_COLS = N_CALLS * (2 * G // 16)  # 256

CODE_LO, CODE_HI = 0x08, 0x10      # fp8 e4m3: 2^-6 and 2^-5
SCALE_M = (4.0, 2.0)               # z premultiplier per parity m
S_OUT = 2.0 ** -4                  # uniform (code * scale) product = 1/16

FP8 = ml_dtypes.float8_e4m3
BF16 = ml_dtypes.bfloat16


def _pack_z(z: np.ndarray):
    """z [K, 64] f32 -> [128, (K/256)*2*128] uint8 fp8 DoubleRow lhsT blocks.
    Group g = c*2 + m holds rows k = 512c + 4p + m + 2i at (partition p,
    sub-row i), scaled by SCALE_M[m]. Cols: [0:64] hi limb, 64 = count
    (SCALE_M[m]), [65:128] lo limb of dims 0..62."""
    K = z.shape[0]
    n512 = K // 512
    p = np.arange(128)[:, None]
    i = np.arange(2)[None, :]
    out = np.empty((2 * n512, 128, 2, 128), dtype=np.uint8)
    for c in range(n512):
        for m in range(2):
            ks = 512 * c + 4 * p + m + 2 * i          # [128, 2]
            zsc = z[ks].astype(np.float32) * SCALE_M[m]  # [128, 2, 64]
            hi8 = zsc.astype(FP8)
            lo8 = (zsc - hi8.astype(np.float32)).astype(FP8)
            blk = np.zeros((128, 2, 128), dtype=np.uint8)
            blk[..., 0] = np.float32(SCALE_M[m]).astype(FP8).view(np.uint8)
            blk[..., 1:65] = hi8.view(np.uint8)
            blk[..., 65:128] = lo8.view(np.uint8)[..., :63]
            out[c * 2 + m] = blk
    return np.ascontiguousarray(out.transpose(1, 0, 2, 3).reshape(128, -1))


def _wrap_idxs(idx: np.ndarray):
    """[n] int -> [128, n/16] int16 wrapped (j -> [j%16, j//16]) + 8x replicated."""
    n = idx.shape[0]
    w = np.zeros((16, n // 16), dtype=np.int16)
    w[np.arange(n) % 16, np.arange(n) // 16] = idx.astype(np.int16)
    return np.tile(w, (8, 1))


def build_body(tc, t):
    """Emit the per-core program. t: dict name -> AP of DRAM tensors."""
    nc = tc.nc
    dt = mybir.dt
    with ExitStack() as ctx:
        const = ctx.enter_context(tc.tile_pool(name="const", bufs=1))
        gpool = ctx.enter_context(tc.tile_pool(name="gath", bufs=2))
        mka = ctx.enter_context(tc.tile_pool(name="mska", bufs=2))
        mkb = ctx.enter_context(tc.tile_pool(name="mskb", bufs=2))
        tails = ctx.enter_context(tc.tile_pool(name="tails", bufs=2))
        pairp = ctx.enter_context(tc.tile_pool(name="pair", bufs=1))
        psum = ctx.enter_context(tc.tile_pool(name="ps", bufs=8, space="PSUM"))

        # idx on the sync queue (gathers depend on it); bulk constants on the
        # scalar HWDGE queue so they don't delay the first gather.
        idxt = const.tile([128, IDX_COLS], dt.int16)
        nc.sync.dma_start(idxt[:], t["IDX"][:, :])

        zd = const.tile([128, 2 * C_A * 2 * 128], dt.uint8)
        nc.scalar.dma_start(zd[:], t["ZD"][:, :])
        zp = const.tile([128, 2 * C_P * 2 * 128], dt.uint8)
        nc.scalar.dma_start(zp[:], t["ZP"][:, :])
        w1t = const.tile([128, 4 * HID], dt.uint16)
        nc.scalar.dma_start(w1t[:], t["W1"][:, :])
        w2t = const.tile([128, 1], dt.uint16)
        nc.scalar.dma_start(w2t[:], t["W2"][:, :])
        b1t = const.tile([128, 1], dt.float32)
        nc.scalar.dma_start(b1t[:], t["B1"][:, :])
        b2t = const.tile([1, 1], dt.float32)
        nc.scalar.dma_start(b2t[:], t["B2"][:, :])

        zsrc = pairp.tile([128, E_LOC], dt.uint16)
        zdst = pairp.tile([128, E_LOC], dt.uint16)

        codes = const.tile([128, 2], dt.uint32)
        nc.vector.memset(codes[:, 0:1], 0x08080808)
        nc.vector.memset(codes[:, 1:2], 0x10101010)

        zd8 = zd[:].bitcast(dt.float8e4).rearrange(
            "p (g two m) -> p g two m", g=2 * C_A, two=2
        )
        zp8 = zp[:].bitcast(dt.float8e4).rearrange(
            "p (g two m) -> p g two m", g=2 * C_P, two=2
        )

        st_state = {}

        def gather_and_mask(et):
            """Gathers + z copies + fp8 mask planes for supertile et."""
            mska_t = mka.tile([128, 2 * C_A * 2 * G * 2 // 2], dt.uint16,
                              tag="a", name=f"mka{et}")
            mskb_t = mkb.tile([128, 2 * C_P * 2 * G * 2 // 2], dt.uint16,
                              tag="b", name=f"mkb{et}")
            oa = mska_t[:].bitcast(dt.uint32).rearrange(
                "p (m c s w) -> p m c s w", m=2, c=C_A, s=2
            )
            ob = mskb_t[:].bitcast(dt.uint32).rearrange(
                "p (m c s w) -> p m c s w", m=2, c=C_P, s=2
            )
            W = G // 2  # u32 words per chunk per endpoint half
            for sub in range(2):
                g = 2 * et + sub
                gt = gpool.tile([128, C_TOT * 2 * G], dt.uint16, tag="gt")
                gv = gt[:].rearrange("p (c i) -> p c i", c=C_TOT)
                nc.gpsimd.dma_gather(
                    out_ap=gv,
                    in_ap=t["A"][:, :],
                    idxs_ap=idxt[:, g * 32:(g + 1) * 32],
                    num_idxs=2 * G,
                    num_idxs_reg=2 * G,
                    elem_size=ROW_U16,
                    elem_step=ROW_U16,
                    transpose=True,
                    single_packet=False,
                    queue_num=0,
                )
                # z chunk -> pair^T rows (s first half, d second half)
                nc.scalar.copy(
                    zsrc[:].bitcast(dt.bfloat16)[:, G * g:G * (g + 1)],
                    gv[:, C_A, 0:G].bitcast(dt.bfloat16),
                )
                nc.scalar.copy(
                    zdst[:].bitcast(dt.bfloat16)[:, G * g:G * (g + 1)],
                    gv[:, C_A, G:2 * G].bitcast(dt.bfloat16),
                )
                g32 = gt[:].bitcast(dt.uint32).rearrange(
                    "p (c w) -> p c w", c=C_TOT
                )
                for m in range(2):
                    nc.vector.scalar_tensor_tensor(
                        oa[:, m, :, sub, :],
                        g32[:, 0:C_A, 0:W],
                        codes[:, m:m + 1],
                        g32[:, 0:C_A, W:2 * W],
                        mybir.AluOpType.bitwise_and,
                        mybir.AluOpType.bitwise_and,
                    )
                    nc.vector.scalar_tensor_tensor(
                        ob[:, m, :, sub, :],
                        g32[:, C_A + 1:C_TOT, 0:W],
                        codes[:, m:m + 1],
                        g32[:, C_A + 1:C_TOT, W:2 * W],
                        mybir.AluOpType.bitwise_and,
                        mybir.AluOpType.bitwise_and,
                    )
            st_state[et] = (mska_t, mskb_t)

        def cn_matmuls(et):
            mska_t, mskb_t = st_state[et]
            psa = psum.tile([128, 512], dt.float32, tag="ps", name=f"psa{et}")
            psb = psum.tile([128, 512], dt.float32, tag="ps", name=f"psb{et}")
            ma = mska_t[:].bitcast(dt.float8e4).rearrange(
                "p (m c i two) -> p c m two i", m=2, c=C_A, two=2
            )
            mb = mskb_t[:].bitcast(dt.float8e4).rearrange(
                "p (m c i two) -> p c m two i", m=2, c=C_P, two=2
            )
            for c in range(C_A):
                for m in range(2):
                    nc.tensor.matmul(
                        psa[:],
                        zd8[:, c * 2 + m],
                        ma[:, c, m],
                        start=(c == 0 and m == 0),
                        stop=(c == C_A - 1 and m == 1),
                        perf_mode=mybir.MatmulPerfMode.DoubleRow,
                    )
            for c in range(C_P):
                for m in range(2):
                    nc.tensor.matmul(
                        psb[:],
                        zp8[:, c * 2 + m],
                        mb[:, c, m],
                        start=(c == 0 and m == 0),
                        stop=(c == C_P - 1 and m == 1),
                        perf_mode=mybir.MatmulPerfMode.DoubleRow,
                    )
            st_state[et] = (psa, psb)

        def tail(et):
            """Normalize + MLP + output for supertile et."""
            psa, psb = st_state.pop(et)
            # counts live in PSUM row 0; clamp+invert them on lane 0 in
            # SBUF (gpsimd cannot read PSUM), then broadcast to all lanes.
            cnt = tails.tile([1, 1024], dt.float32, tag="cnt", bufs=1)
            nc.vector.tensor_scalar_max(cnt[0:1, 0:512], psa[0:1, :], S_OUT)
            nc.vector.tensor_scalar_max(cnt[0:1, 512:1024], psb[0:1, :], S_OUT)
            rec = tails.tile([1, 1024], dt.float32, tag="rec", bufs=1)
            nc.vector.reciprocal_approx_fast(rec[:], cnt[:])
            rhs = {}
            for rel, ps, c0 in (("a", psa, 0), ("b", psb, 512)):
                bc = tails.tile([128, 512], dt.float32, tag=f"bc{rel}")
                nc.gpsimd.partition_broadcast(bc[:], rec[0:1, c0:c0 + 512])
                rh = tails.tile([128, 512], dt.bfloat16, tag=f"rh{rel}")
                nc.vector.tensor_tensor(
                    rh[:], ps[:], bc[:], mybir.AluOpType.mult
                )
                rhs[rel] = rh
            hps = psum.tile([HID, 512], dt.float32, tag="ps")
            rhs_chunks = (
                zsrc[:].bitcast(dt.bfloat16)[:, 512 * et:512 * (et + 1)],
                zdst[:].bitcast(dt.bfloat16)[:, 512 * et:512 * (et + 1)],
                rhs["a"][:],
                rhs["b"][:],
            )
            for j, r in enumerate(rhs_chunks):
                nc.tensor.matmul(
                    hps[:],
                    w1t[:].bitcast(dt.bfloat16)[:, HID * j:HID * (j + 1)],
                    r,
                    start=(j == 0),
                    stop=(j == 3),
                )
            hsb = tails.tile([HID, 512], dt.bfloat16, tag="h", bufs=1)
            nc.scalar.activation(
                hsb[:], hps[:], mybir.ActivationFunctionType.Relu, bias=b1t[:, 0:1]
            )
            lps = psum.tile([1, 512], dt.float32, tag="ps")
            nc.tensor.matmul(
                lps[:], w2t[:].bitcast(dt.bfloat16), hsb[:], start=True, stop=True
            )
            osb = tails.tile([1, 512], dt.float32, tag="osb", bufs=1)
            nc.scalar.activation(
                osb[:],
                lps[:],
                mybir.ActivationFunctionType.Sigmoid,
                bias=b2t[:, 0:1],
            )
            nc.sync.dma_start(t["OUT"][:, 512 * et:512 * (et + 1)], osb[:])

        for et in range(N_ET):
            gather_and_mask(et)
            cn_matmuls(et)
            if et > 0:
                tail(et - 1)
        tail(N_ET - 1)


def build_program():
    nc = bacc.Bacc(
        "TRN2",
        target_bir_lowering=False,
        num_swdge_queues=1,
        dynamic_dma_scratch_size=32768,
    )
    dt = mybir.dt
    t = {
        "A": nc.dram_tensor("A", [U_PAD, ROW_U16], dt.uint16, kind="ExternalInput").ap(),
        "IDX": nc.dram_tensor("IDX", [128, IDX_COLS], dt.int16, kind="ExternalInput").ap(),
        "ZD": nc.dram_tensor(
            "ZD", [128, 2 * C_A * 2 * 128], dt.uint8, kind="ExternalInput"
        ).ap(),
        "ZP": nc.dram_tensor(
            "ZP", [128, 2 * C_P * 2 * 128], dt.uint8, kind="ExternalInput"
        ).ap(),
        "W1": nc.dram_tensor("W1", [128, 4 * HID], dt.uint16, kind="ExternalInput").ap(),
        "B1": nc.dram_tensor("B1", [HID, 1], dt.float32, kind="ExternalInput").ap(),
        "W2": nc.dram_tensor("W2", [HID, 1], dt.uint16, kind="ExternalInput").ap(),
        "B2": nc.dram_tensor("B2", [1, 1], dt.float32, kind="ExternalInput").ap(),
        "OUT": nc.dram_tensor("OUT", [1, E_LOC], dt.float32, kind="ExternalOutput").ap(),
    }
    with tile.TileContext(nc) as tc:
        build_body(tc, t)
    nc.compile()
    return nc


def host_prep(z_drug, z_protein, ddi_ei, dp_ei, pred_ei, W1, b1, W2, b2):
    """Build the 8 per-core input maps (all numpy, no device work)."""
    z_drug = np.asarray(z_drug, np.float32)
    z_protein = np.asarray(z_protein, np.float32)
    ddi_ei = np.asarray(ddi_ei, np.int64)
    dp_ei = np.asarray(dp_ei, np.int64)
    pred_ei = np.asarray(pred_ei, np.int64)

    A_ddi = np.zeros((N_D, N_D), dtype=np.uint8)
    A_ddi[ddi_ei[0], ddi_ei[1]] = 1
    A_ddi[ddi_ei[1], ddi_ei[0]] = 1
    A_dp = np.zeros((N_D, N_P), dtype=np.uint8)
    A_dp[dp_ei[0], dp_ei[1]] = 1
    # bit-coded nibble pack: entry 2j -> 0x08 (low), entry 2j+1 -> 0x10 (high)
    A_ddi_nib = (A_ddi[:, 0::2] * CODE_LO) | (A_ddi[:, 1::2] * CODE_HI)
    A_dp_nib = (A_dp[:, 0::2] * CODE_LO) | (A_dp[:, 1::2] * CODE_HI)

    zb_full = np.zeros((N_D, 128), dtype=np.float32)
    zb_full[:, :D_DIM] = z_drug
    zb_bytes = zb_full.astype(BF16).view(np.uint8)  # [N_D, 256]

    ZD = _pack_z(z_drug)
    ZP = _pack_z(z_protein)

    # W1 blocks [128, HID] bf16: rows 0:64 = hi-limb dims, row 64 = 0 (count
    # row), rows 65:128 = lo-limb dims 0..62 (duplicated weights sum limbs).
    W1f = np.asarray(W1, np.float32)  # [256, HID]
    blocks = np.zeros((4, 128, HID), dtype=np.float32)
    blocks[0, 0:64] = W1f[0:64]
    blocks[1, 0:64] = W1f[64:128]
    blocks[2, 1:65] = W1f[128:192]
    blocks[2, 65:128] = W1f[128:191]
    blocks[3, 1:65] = W1f[192:256]
    blocks[3, 65:128] = W1f[192:255]
    W1p = np.ascontiguousarray(
        blocks.astype(BF16).view(np.uint16).transpose(1, 0, 2).reshape(128, 4 * HID)
    )
    B1 = np.asarray(b1, np.float32).reshape(HID, 1)
    W2p = np.asarray(W2, np.float32).reshape(HID, 1).astype(BF16).view(np.uint16)
    B2 = np.asarray(b2, np.float32).reshape(1, 1)

    in_maps = []
    for core in range(N_CORES):
        s = pred_ei[0, core * E_LOC:(core + 1) * E_LOC]
        d = pred_ei[1, core * E_LOC:(core + 1) * E_LOC]
        rows = np.unique(np.concatenate([s, d]))
        nu = rows.shape[0]
        assert nu <= U_PAD
        remap_s = np.searchsorted(rows, s).astype(np.int16)
        remap_d = np.searchsorted(rows, d).astype(np.int16)
        A = np.zeros((U_PAD, 2 * ROW_U16), dtype=np.uint8)
        A[:nu, 0:N_D // 2] = A_ddi_nib[rows]
        A[:nu, N_D // 2:N_D // 2 + 256] = zb_bytes[rows]
        A[:nu, N_D // 2 + 256:] = A_dp_nib[rows]

        cols = []
        for g in range(N_CALLS):
            cols.append(
                _wrap_idxs(
                    np.concatenate(
                        [remap_s[G * g:G * (g + 1)], remap_d[G * g:G * (g + 1)]]
                    )
                )
            )
        idx = np.concatenate(cols, axis=1)
        assert idx.shape == (128, IDX_COLS)

        in_maps.append(
            {
                "A": A.view(np.uint16),
                "IDX": idx,
                "ZD": ZD,
                "ZP": ZP,
                "W1": W1p,
                "B1": B1,
                "W2": W2p,
                "B2": B2,
            }
        )
    return in_maps


def kernel(z_drug, z_protein, ddi_ei, dp_ei, pred_ei, W1, b1, W2, b2, _profile=None):
    from concourse.bass_utils import run_bass_kernel_spmd

    in_maps = host_prep(z_drug, z_protein, ddi_ei, dp_ei, pred_ei, W1, b1, W2, b2)
    nc = build_program()
    res = run_bass_kernel_spmd(
        nc,
        in_maps,
        core_ids=list(range(N_CORES)),
        **({} if _profile is None else _profile),
    )
    if _profile is not None:
        kernel.last_results = res
    out = np.concatenate([r["OUT"].reshape(-1) for r in res.results])
    return out.astype(np.float32)
